# revision 27
# baseline (speedup 1.0000x reference)
"""AttentionWithFastKAN Trainium2 kernel (v3).

Strategy (8 NeuronCores, data-parallel over batch):
  - Each core processes one batch element (1024 tokens) end to end.
  - FastKAN: channel-major activations (c*g on partitions).  RBF basis via
    Derivative_Erf(u) = 2/sqrt(pi)*exp(-u^2) on ScalarE; sqrt(pi)/2 folded
    into spline weights host-side.  LayerNorm stats via ones-matmuls on PE
    (partition reduction), accumulated into the spare padding columns of the
    v/proj psum tiles so no extra PSUM pool serializes phase starts.
  - Precision split: Q/K spline path f32r (peaked softmax amplifies qkv
    error ~8x); V and proj spline paths bf16.
  - Attention (rebuilt vs v2): per head-pair, per key-tile pipeline
    S^T -> exp -> A@V accumulate.  The softmax denominator of the
    row-0..63 head rides as a 65th ones-column in the V stationary
    operand; the row-64..127 head's denominator accumulates via a
    ones-matmul into spare partition 96 of the partner psum tile.
    Normalization reciprocals are broadcast across partitions with K=1
    ones-matmuls on the PE into the partner tile's unused partition
    ranges -- attention uses exactly 8 PSUM banks, no gpsimd, and the
    exp (ScalarE) runs concurrent with S/AV matmuls (PE).
  - proj: token-major output => contiguous output DMA.
"""

import math

import numpy as np
import ml_dtypes

import concourse.bass as bass
import concourse.mybir as mybir
import concourse.tile as tile
from concourse import bacc
from concourse.bass_utils import run_bass_kernel_spmd

F32 = mybir.dt.float32
F32R = mybir.dt.float32r
BF16 = mybir.dt.bfloat16
AF = mybir.ActivationFunctionType

B, N_TOK, C = 8, 1024, 768
G = 8
H = 12
CT = C // 128               # 6 channel ptiles
KT = CT * G + CT            # 54 contraction tiles (48 spline + 6 base)
GRID = np.linspace(-2.0, 2.0, G).astype(np.float64)
DENOM = 4.0 / 7.0
SQPI2 = math.sqrt(math.pi) / 2.0

# contraction order: silu (base) tiles first, then spline tiles
K_ORDER = list(range(CT * G, KT)) + list(range(CT * G))


def build_kernel(T=1024, sim_safe=False, debug_out=False):
    TT = T // 128                       # token ptiles
    af_silu = AF.Sigmoid if sim_safe else AF.Silu
    af_derf = AF.Square if sim_safe else AF.Derivative_Erf

    nc = bacc.Bacc("TRN2", target_bir_lowering=False, debug=False, num_devices=8)

    # ---- dram io ----
    xT_d = nc.dram_tensor("xT", (C, T), F32R, kind="ExternalInput")
    w1qk_d = nc.dram_tensor("w1qk", (KT, 128, 1536), F32R, kind="ExternalInput")
    w1v_d = nc.dram_tensor("w1v", (KT, 128, 768), BF16, kind="ExternalInput")
    w2_d = nc.dram_tensor("w2", (KT, 128, 768), BF16, kind="ExternalInput")
    b1qk_d = nc.dram_tensor("b1qk", (128, 12), F32, kind="ExternalInput")
    b1v_d = nc.dram_tensor("b1v", (1, 768), F32, kind="ExternalInput")
    b2_d = nc.dram_tensor("b2", (1, 768), F32, kind="ExternalInput")
    asc1_d = nc.dram_tensor("asc1", (128, CT), F32, kind="ExternalInput")
    abi1_d = nc.dram_tensor("abi1", (128, CT * G), F32, kind="ExternalInput")
    asc2_d = nc.dram_tensor("asc2", (128, CT), F32, kind="ExternalInput")
    abi2_d = nc.dram_tensor("abi2", (128, CT * G), F32, kind="ExternalInput")
    out_d = nc.dram_tensor("out", (T, C), F32, kind="ExternalOutput")
    if debug_out:
        dbg_qkT = nc.dram_tensor("dbg_qkT", (128, 12, T), F32, kind="ExternalOutput")
        dbg_V4e = nc.dram_tensor("dbg_V4e", (128, T // 128, 12, 65), mybir.dt.bfloat16, kind="ExternalOutput")
        dbg_OT = nc.dram_tensor("dbg_OT", (128, CT, T), F32, kind="ExternalOutput")
        dbg_h1 = nc.dram_tensor("dbg_h1", (128, CT, T), F32, kind="ExternalOutput")
        dbg_rrt = nc.dram_tensor("dbg_rrt", (6, 128, T), F32, kind="ExternalOutput")

    with tile.TileContext(nc) as tc:
        with tc.tile_pool(name="const", bufs=1) as const, \
             tc.tile_pool(name="potp", bufs=1) as potp:

            # ---- constants ----
            asc1 = const.tile([128, CT], F32)
            abi1 = const.tile([128, CT * G], F32)
            asc2 = const.tile([128, CT], F32)
            abi2 = const.tile([128, CT * G], F32)
            nc.sync.dma_start(asc1[:], asc1_d[:])
            nc.sync.dma_start(abi1[:], abi1_d[:])
            nc.sync.dma_start(asc2[:], asc2_d[:])
            nc.sync.dma_start(abi2[:], abi2_d[:])
            b1qk = const.tile([128, 12], F32)
            nc.sync.dma_start(b1qk[:], b1qk_d[:])
            b1v_row = const.tile([1, 768], F32)
            b2_row = const.tile([1, 768], F32)
            nc.sync.dma_start(b1v_row[:], b1v_d[:])
            nc.sync.dma_start(b2_row[:], b2_d[:])
            b1v_b = const.tile([128, 768], F32)
            b2_b = const.tile([128, 768], F32)
            nc.gpsimd.partition_broadcast(b1v_b[:], b1v_row[:])
            nc.gpsimd.partition_broadcast(b2_b[:], b2_row[:])
            ones_f32 = const.tile([128, 1], F32)
            nc.vector.memset(ones_f32[:], 1.0)
            ones_f_t = const.tile([128, 1], F32R)
            nc.vector.tensor_copy(ones_f_t[:], ones_f32[:])
            ones_f = ones_f_t[:]
            ones_bf = const.tile([128, 1], BF16)
            nc.vector.memset(ones_bf[:], 1.0)
            sel32 = const.tile([128, 64], BF16)
            sel64 = const.tile([128, 64], BF16)
            nc.vector.memset(sel32[:], 0.0)
            nc.vector.memset(sel64[:], 0.0)
            nc.vector.memset(sel32[32:33, :], 1.0)
            nc.vector.memset(sel64[64:65, :], 1.0)
            eps_t = const.tile([1, 1], F32)
            nc.vector.memset(eps_t[:], 1e-5)

            # ---- persistent activations ----
            qkT = potp.tile([128, 12, T], F32R)     # q,k channel-major
            V4e = potp.tile([128, TT, 12, 65], BF16)  # v token-major + ones col
            nc.gpsimd.memset(V4e[:, :, :, 64], 1.0)

            def ln_finalize(sget, ssget, tmp1, big):
                """Finalize LN stats from psum chunk getters -> broadcast
                rs_b/murs_b [128,T]."""
                mean = tmp1.tile([1, T], F32, tag="st_mean")
                bv = tmp1.tile([1, T], F32, tag="st_bv")
                cv = tmp1.tile([1, T], F32, tag="st_cv")
                nc.vector.tensor_scalar_mul(mean[:], sget, 1.0 / C)
                nc.vector.tensor_scalar_mul(bv[:], ssget, 1.0 / C)
                nc.vector.tensor_mul(cv[:], mean[:], mean[:])
                nc.vector.tensor_sub(bv[:], bv[:], cv[:])
                # 1/sqrt(var + eps): Sqrt activation + fast NR reciprocal
                nc.scalar.activation(out=bv[:], in_=bv[:], func=AF.Sqrt,
                                     bias=eps_t[:], scale=1.0)
                scr = tmp1.tile([1, T], F32, tag="st_scr")
                nc.vector.reciprocal_approx_accurate(bv[:], bv[:], scr[:])
                nc.vector.tensor_mul(cv[:], mean[:], bv[:])
                rs_b = big.tile([128, T], F32, tag="rs_b")
                murs_b = big.tile([128, T], F32, tag="murs_b")
                nc.gpsimd.partition_broadcast(rs_b[:], bv[:])
                nc.gpsimd.partition_broadcast(murs_b[:], cv[:])
                return rs_b, murs_b

            def stat_mms(st_s, st_ss, src_f32r, sq_f32r, start, stop):
                """Accumulate sum/sumsq of one [128,T] ct tile into row 0 of
                two [128,T] psum tiles (chunked 512 for bank alignment)."""
                for ch in range(2):
                    sl = slice(ch * 512, (ch + 1) * 512)
                    nc.tensor.matmul(st_s[0:1, sl], ones_f,
                                     src_f32r[:, sl], start=start, stop=stop)
                    nc.tensor.matmul(st_ss[0:1, sl], ones_f,
                                     sq_f32r[:, sl], start=start, stop=stop)

            def make_h(src_f32, rs_b, murs_b, big, tag="hT"):
                hT = big.tile([128, CT, T], F32, tag=tag)
                for ct in range(CT):
                    nc.vector.tensor_mul(hT[:, ct], src_f32(ct), rs_b[:])
                    nc.vector.tensor_sub(hT[:, ct], hT[:, ct], murs_b[:])
                return hT

            def basis_tile(hT, siluT, k, tok0, width, pool, asc, abi, dt):
                """[128, width] contraction tile k (basis or silu slice)."""
                if k < CT * G:
                    ct = k % CT
                    bt = pool.tile([128, width], dt, tag="basis")
                    nc.scalar.activation(out=bt[:],
                                         in_=hT[:, ct, tok0:tok0 + width],
                                         func=af_derf,
                                         scale=asc[:, ct:ct + 1],
                                         bias=abi[:, k:k + 1])
                    return bt[:]
                ct = k - CT * G
                return siluT[:, ct, tok0:tok0 + width]

            # ================= layer 1 =================
            ln1big = tc.tile_pool(name="ln1big", bufs=1)
            ln1 = ln1big.__enter__()
            hT1 = None
            siluT1 = ln1.tile([128, CT, T], F32R, tag="siluT")
            siluT1b = ln1.tile([128, CT, T], BF16, tag="siluTb")

            ln1bc = tc.tile_pool(name="ln1bc", bufs=1)
            ln1bcp = ln1bc.__enter__()

            xpool = tc.tile_pool(name="xload", bufs=1)
            xp = xpool.__enter__()
            xT = xp.tile([128, CT, T], F32R)
            _qs = (nc.sync, nc.scalar, nc.gpsimd)
            for ct in range(CT):
                _qs[ct % 3].dma_start(
                    xT[:, ct],
                    xT_d.rearrange("(ct p) t -> ct p t", p=128)[ct])
            # silu path: only needs x -- emitted first so PE base matmuls
            # can start while the LN chain is still in flight.
            for ct in range(CT):
                nc.scalar.activation(out=siluT1[:, ct], in_=xT[:, ct].bitcast(F32),
                                     func=af_silu)

            # weight pools open before LN temp pools so their SBUF ranges
            # don't alias -- weight DMAs stream during the LN chain.
            w1vs_cm = tc.tile_pool(name="w1vs", bufs=8)
            w1vs = w1vs_cm.__enter__()
            bas1v_cm = tc.tile_pool(name="bas1v", bufs=6)
            bas1v = bas1v_cm.__enter__()

            # ---- LN1 stats (dedicated psum pool, closes before v) ----
            with tc.tile_pool(name="ln1tmp", bufs=2) as ln1tmp, \
                 tc.tile_pool(name="ln1tmp1", bufs=1) as ln1tmp1, \
                 tc.tile_pool(name="ps_st1", bufs=1, space="PSUM") as ps_st1:
                st_s = ps_st1.tile([128, T], F32, tag="st_s")
                st_ss = ps_st1.tile([128, T], F32, tag="st_ss")
                for ct in range(CT):
                    xsq = ln1tmp.tile([128, T], F32R, tag="xsq")
                    nc.vector.tensor_mul(xsq[:], xT[:, ct].bitcast(F32),
                                         xT[:, ct].bitcast(F32))
                    stat_mms(st_s, st_ss, xT[:, ct], xsq[:],
                             start=(ct == 0), stop=(ct == CT - 1))
                rs_b, murs_b = ln_finalize(st_s[0:1, :], st_ss[0:1, :],
                                           ln1tmp1, ln1bcp)
            for ct in range(CT):
                nc.gpsimd.tensor_copy(siluT1b[:, ct],
                                      siluT1[:, ct].bitcast(F32))
            hT1 = make_h(lambda ct: xT[:, ct].bitcast(F32), rs_b, murs_b,
                         ln1)

            # ---- v phase ----
            with tc.tile_pool(name="ps_v", bufs=4, space="PSUM") as ps_v:
                for tp in range(2):
                    tts = range(4 * tp, 4 * tp + 4)
                    tok0 = 4 * tp * 128
                    psum = {tt: ps_v.tile([128, 768], F32, tag="psv",
                                          name=f"psv_{tp}_{tt}")
                            for tt in tts}
                    for ki, k in enumerate(K_ORDER):
                        wt = w1vs.tile([128, 768], BF16, tag="w1vt",
                                       name=f"w1vt_{tp}_{ki}")
                        nc.sync.dma_start(wt[:], w1v_d[k])
                        bt = basis_tile(hT1, siluT1b, k, tok0, 512,
                                        bas1v, asc1, abi1, BF16)
                        for i, tt in enumerate(tts):
                            lhs = bt[:, i * 128:(i + 1) * 128]
                            nc.tensor.matmul(
                                psum[tt][:, 0:512], lhs, wt[:, 0:512],
                                start=(ki == 0), stop=(ki == KT - 1))
                            nc.tensor.matmul(
                                psum[tt][:, 512:768], lhs, wt[:, 512:768],
                                start=(ki == 0), stop=(ki == KT - 1))
                    for tt in tts:
                        nc.vector.tensor_add(
                            V4e[:, tt, :, 0:64],
                            psum[tt][:, 0:768].rearrange(
                                "p (h c) -> p h c", h=12),
                            b1v_b[:].rearrange("p (h c) -> p h c", h=12))

            bas1v_cm.__exit__(None, None, None)
            w1vs_cm.__exit__(None, None, None)
            xpool.__exit__(None, None, None)

            if debug_out:
                nc.sync.dma_start(dbg_h1[:], hT1[:])
                nc.sync.dma_start(dbg_V4e[:], V4e[:])

            # ---- q,k: weights stationary (f32r), basis moving (f32r) ----
            with tc.tile_pool(name="w1s", bufs=8) as w1s, \
                 tc.tile_pool(name="bas1", bufs=6) as bas1, \
                 tc.tile_pool(name="ps_qk", bufs=4, space="PSUM") as ps_qk:
                for ots in (range(0, 4), range(4, 8), range(8, 12)):
                    psum = {ot: ps_qk.tile([128, T], F32, tag="psqk",
                                           name=f"psqk_{ot}")
                            for ot in ots}
                    for ki, k in enumerate(K_ORDER):
                        wt = w1s.tile([128, 512], F32R, tag="w1t")
                        nc.sync.dma_start(
                            wt[:], w1qk_d[k, :, ots[0] * 128:(ots[-1] + 1) * 128])
                        bt = basis_tile(hT1, siluT1, k, 0, T,
                                        bas1, asc1, abi1, F32R)
                        for j, ot in enumerate(ots):
                            lhs = wt[:, j * 128:(j + 1) * 128]
                            for ch in range(2):
                                nc.tensor.matmul(
                                    psum[ot][:, ch * 512:(ch + 1) * 512],
                                    lhs, bt[:, ch * 512:(ch + 1) * 512],
                                    start=(ki == 0), stop=(ki == KT - 1))
                            if ki == KT - 1:
                                # evacuate as soon as this ot's accumulation
                                # stops so the psum pool closes right behind
                                # the last matmul (keeps HAM warm into the
                                # next phase)
                                nc.vector.tensor_scalar_add(
                                    qkT[:, ot], psum[ot][:],
                                    b1qk[:, ot:ot + 1])
                    if ots[-1] == 11:
                        # dummy matmuls bridge the psum pool swap so the PE
                        # never idles past the HAM window into attention
                        for dummy in range(5):
                            nc.tensor.matmul(
                                psum[ots[-1]][0:1, 0:512], ones_f,
                                qkT[:, 0, 0:512], start=True, stop=True,
                                skip_group_check=True)

            if debug_out:
                nc.sync.dma_start(dbg_qkT[:], qkT[:].bitcast(F32))

            ln1bc.__exit__(None, None, None)
            ln1big.__exit__(None, None, None)

            # ================= attention =================
            ot_pool = tc.tile_pool(name="otp", bufs=1)
            otp = ot_pool.__enter__()
            OT = otp.tile([128, CT, T], F32R)      # attn out channel-major
            osq_pool = tc.tile_pool(name="osqp", bufs=1)
            osqp = osq_pool.__enter__()
            OSQ = osqp.tile([128, CT, T], F32R)    # OT^2 for LN2 stats

            with tc.tile_pool(name="attnp", bufs=4) as attnp, \
                 tc.tile_pool(name="attn_sm", bufs=2) as attn_sm, \
                 tc.tile_pool(name="ps_at", bufs=1, space="PSUM") as ps_at:
                for hp in range(H // 2):
                    hA, hB = 2 * hp, 2 * hp + 1
                    q_ot, k_ot = hp, 6 + hp
                    dent = attn_sm.tile([128, T], F32, tag="dent",
                                        name=f"dent_{hp}")
                    avts = {}
                    for ch in range(2):
                        sl = slice(ch * 512, (ch + 1) * 512)
                        poA = ps_at.tile([128, 512], F32, tag="po", bufs=2,
                                         name=f"poA_{hp}_{ch}")
                        poB = ps_at.tile([128, 512], F32, tag="po", bufs=2,
                                         name=f"poB_{hp}_{ch}")
                        ET = {}

                        def emit_av(mt, hA=hA, hB=hB, poA=poA, poB=poB,
                                    ET=ET):
                            st, sp = (mt == 0), (mt == TT - 1)
                            nc.tensor.matmul(
                                poA[0:65], V4e[:, mt, hA, 0:65],
                                ET[("A", mt)][:],
                                start=st, stop=sp, skip_group_check=True)
                            nc.tensor.matmul(
                                poB[64:128], V4e[:, mt, hB, 0:64],
                                ET[("B", mt)][:],
                                start=st, stop=sp)
                            nc.tensor.matmul(
                                poB[32:33], ones_bf[:], ET[("B", mt)][:],
                                start=st, stop=sp, skip_group_check=True)

                        for mt in range(TT):
                            for j, tg in enumerate(("A", "B")):
                                bp = 64 * j
                                ps = ps_at.tile(
                                    [128, 512], F32, tag="s", bufs=6,
                                    name=f"s_{hp}_{ch}_{mt}_{tg}")
                                nc.tensor.matmul(
                                    ps[:],
                                    qkT[bp:bp + 64, k_ot,
                                        mt * 128:(mt + 1) * 128],
                                    qkT[bp:bp + 64, q_ot, sl],
                                    start=True, stop=True)
                                et = attnp.tile(
                                    [128, 512], BF16, tag="et", bufs=8,
                                    name=f"et_{hp}_{ch}_{mt}_{tg}")
                                nc.scalar.activation(out=et[:], in_=ps[:],
                                                     func=AF.Exp,
                                                     scale=0.125)
                                ET[(tg, mt)] = et
                            if mt >= 1:
                                emit_av(mt - 1)
                        emit_av(TT - 1)

                        # evacuate AV + denominator rows (releases po)
                        avt = attn_sm.tile([128, 512], F32, tag="avt",
                                           bufs=4, name=f"avt_{hp}_{ch}")
                        nc.vector.tensor_copy(avt[0:64], poA[0:64])
                        nc.vector.tensor_copy(avt[64:128], poB[64:128])
                        nc.vector.tensor_copy(dent[64:65, sl], poA[64:65])
                        nc.vector.tensor_copy(dent[32:33, sl], poB[32:33])
                        avts[ch] = avt

                    # ---- normalize (all off the PE queue) ----
                    denA = attn_sm.tile([128, T], F32, tag="denA",
                                        name=f"denA_{hp}")
                    denB = attn_sm.tile([128, T], F32, tag="denB",
                                        name=f"denB_{hp}")
                    nc.sync.dma_start(denA[0:1, :], dent[64:65, :])
                    nc.sync.dma_start(denB[0:1, :], dent[32:33, :])
                    nc.vector.reciprocal_approx_fast(denA[0:1, :],
                                                     denA[0:1, :])
                    nc.vector.reciprocal_approx_fast(denB[0:1, :],
                                                     denB[0:1, :])
                    rbA = attn_sm.tile([128, T], F32, tag="rbA",
                                       name=f"rbA_{hp}")
                    rbB = attn_sm.tile([128, T], F32, tag="rbB",
                                       name=f"rbB_{hp}")
                    nc.gpsimd.partition_broadcast(rbA[:], denA[0:1, :])
                    nc.gpsimd.partition_broadcast(rbB[:], denB[0:1, :])
                    for ch in range(2):
                        sl = slice(ch * 512, (ch + 1) * 512)
                        nc.vector.tensor_mul(OT[0:64, hp, sl],
                                             avts[ch][0:64], rbA[0:64, sl])
                        nc.vector.tensor_mul(OT[64:128, hp, sl],
                                             avts[ch][64:128],
                                             rbB[64:128, sl])
                    nc.vector.tensor_mul(OSQ[:, hp], OT[:, hp].bitcast(F32),
                                         OT[:, hp].bitcast(F32))
                    if debug_out:
                        nc.sync.dma_start(dbg_rrt[hp], dent[:])
            if debug_out:
                nc.sync.dma_start(dbg_OT[:], OT[:].bitcast(F32))

            # ================= layer 2 (proj, bf16) =================
            with tc.tile_pool(name="ln2big", bufs=1) as ln2big:
                siluT2 = ln2big.tile([128, CT, T], BF16, tag="siluT2")
                for ct in range(CT):
                    nc.scalar.activation(out=siluT2[:, ct],
                                         in_=OT[:, ct].bitcast(F32),
                                         func=af_silu)

                with tc.tile_pool(name="ln2tmp1", bufs=1) as ln2tmp1, \
                     tc.tile_pool(name="ps_st2", bufs=1, space="PSUM") as ps_st2:
                    st_s2 = ps_st2.tile([128, T], F32, tag="st_s2")
                    st_ss2 = ps_st2.tile([128, T], F32, tag="st_ss2")
                    for ct in range(CT):
                        stat_mms(st_s2, st_ss2, OT[:, ct], OSQ[:, ct],
                                 start=(ct == 0), stop=(ct == CT - 1))
                    rs_b2, murs_b2 = ln_finalize(st_s2[0:1, :],
                                                 st_ss2[0:1, :],
                                                 ln2tmp1, ln2big)
                    hT2 = make_h(lambda ct: OT[:, ct].bitcast(F32),
                                 rs_b2, murs_b2, ln2big, tag="hT2")

                with tc.tile_pool(name="w2s", bufs=8) as w2s, \
                     tc.tile_pool(name="bas2", bufs=6) as bas2, \
                     tc.tile_pool(name="outst", bufs=3) as outst, \
                     tc.tile_pool(name="ps_p", bufs=4, space="PSUM") as ps_p:
                    for tp in range(2):
                        tts = range(4 * tp, 4 * tp + 4)
                        tok0 = 4 * tp * 128
                        psum = {tt: ps_p.tile([128, 768], F32, tag="psp",
                                              name=f"psp_{tp}_{tt}")
                                for tt in tts}
                        for ki, k in enumerate(K_ORDER):
                            wt = w2s.tile([128, 768], BF16, tag="w2t",
                                          name=f"w2t_{tp}_{ki}")
                            nc.sync.dma_start(wt[:], w2_d[k])
                            bt = basis_tile(hT2, siluT2, k, tok0, 512,
                                            bas2, asc2, abi2, BF16)
                            for i, tt in enumerate(tts):
                                lhs = bt[:, i * 128:(i + 1) * 128]
                                nc.tensor.matmul(
                                    psum[tt][:, 0:512], lhs, wt[:, 0:512],
                                    start=(ki == 0), stop=(ki == KT - 1))
                                nc.tensor.matmul(
                                    psum[tt][:, 512:768], lhs, wt[:, 512:768],
                                    start=(ki == 0), stop=(ki == KT - 1))
                        for tt in tts:
                            ob = outst.tile([128, 768], F32, tag="ob")
                            nc.vector.tensor_add(ob[:], psum[tt][:, 0:768],
                                                 b2_b[:])
                            _oqs = (nc.sync, nc.scalar, nc.gpsimd)
                            _oqs[tt % 3].dma_start(
                                out_d.rearrange("(tt p) o -> tt p o", p=128)[tt],
                                ob[:])

            osq_pool.__exit__(None, None, None)
            ot_pool.__exit__(None, None, None)

    nc.compile()
    return nc


def host_prep(inputs, T=1024):
    """Build per-core input maps from the full (unsharded) inputs."""
    x = np.asarray(inputs["x"], dtype=np.float32)

    def pack_layer(spline_w, base_w, ln_w, ln_b):
        spline_w = np.asarray(spline_w, dtype=np.float64)
        base_w = np.asarray(base_w, dtype=np.float64)
        O = spline_w.shape[1]
        W = np.empty((KT, 128, O), dtype=np.float64)
        for g in range(G):
            sg = spline_w[g::G] * SQPI2          # [768, O]
            for ct in range(CT):
                W[g * CT + ct] = sg[ct * 128:(ct + 1) * 128]
        for ct in range(CT):
            W[CT * G + ct] = base_w[ct * 128:(ct + 1) * 128]
        ln_w = np.asarray(ln_w, dtype=np.float64)
        ln_b = np.asarray(ln_b, dtype=np.float64)
        asc = (ln_w / DENOM).reshape(CT, 128).astype(np.float32)
        abi = np.empty((CT * G, 128), dtype=np.float32)
        for g in range(G):
            for ct in range(CT):
                abi[g * CT + ct] = \
                    ((ln_b - GRID[g]) / DENOM)[ct * 128:(ct + 1) * 128]
        return W, asc, abi

    W1, asc1, abi1 = pack_layer(inputs["qkv_spline_w"], inputs["qkv_base_w"],
                                inputs["qkv_ln_w"], inputs["qkv_ln_b"])
    W2, asc2, abi2 = pack_layer(inputs["proj_spline_w"], inputs["proj_base_w"],
                                inputs["proj_ln_w"], inputs["proj_ln_b"])
    b1 = np.asarray(inputs["qkv_base_b"], dtype=np.float32)
    b2 = np.asarray(inputs["proj_base_b"], dtype=np.float32)

    shared = {
        "w1qk": np.ascontiguousarray(W1[:, :, :1536]).astype(np.float32),
        "w1v": np.ascontiguousarray(W1[:, :, 1536:]).astype(ml_dtypes.bfloat16),
        "w2": np.ascontiguousarray(W2).astype(ml_dtypes.bfloat16),
        "b1qk": np.ascontiguousarray(b1[:1536].reshape(12, 128).T),
        "b1v": b1[1536:].reshape(1, 768).copy(),
        "b2": b2.reshape(1, 768).copy(),
        "asc1": np.ascontiguousarray(asc1.T),
        "abi1": np.ascontiguousarray(abi1.T),
        "asc2": np.ascontiguousarray(asc2.T),
        "abi2": np.ascontiguousarray(abi2.T),
    }
    in_maps = []
    for core in range(x.shape[0]):
        m = dict(shared)
        m["xT"] = np.ascontiguousarray(x[core, :T].T)
        in_maps.append(m)
    return in_maps


_NC_CACHE = {}


def _get_nc(T=1024):
    if T not in _NC_CACHE:
        _NC_CACHE[T] = build_kernel(T)
    return _NC_CACHE[T]


def kernel(**inputs) -> np.ndarray:
    nc = _get_nc()
    in_maps = host_prep(inputs)
    res = run_bass_kernel_spmd(nc, in_maps, core_ids=list(range(8)))
    out = np.stack([res.results[c]["out"] for c in range(len(in_maps))])
    return out.astype(np.float32)


if __name__ == "__main__":
    data = np.load("/root/problem/ref_data.npz")
    inputs = {k[3:]: data[k] for k in data.files if k.startswith("in_")}
    expected = data["expected64"]
    actual = kernel(**inputs)
    err = np.abs(actual - expected)
    print("absmax err:", err.max(),
          "rel2max:", err.max() / np.abs(expected).max())
    print("rel l2:",
          np.linalg.norm(actual - expected) / np.linalg.norm(expected))
